# revision 4
# baseline (speedup 1.0000x reference)
"""Causal multi-head self-attention on 8 TRN2 NeuronCores.

Sharding: core = (batch b, head-group g): 4 batches x 2 groups of 8 heads.
Host pre-transposes all operands so every TensorE matmul contracts over the
partition dim with zero on-device transposes:

  qkv:      qk^T[n, i]  = sum_k Wqk[n, k] xT[k, i]      (lhsT=WqkT blk, rhs=xT)
            v[j, n]     = sum_k xT[k, j] WvT[k, n]      (lhsT=xT blk,   rhs=WvT)
  attn (per head, per 512-wide i-chunk, per 128-deep j-block):
            S^T[j, i]   = sum_d kT[d, j] qT[d, i]       (lhsT=kT blk,   rhs=qT)
            A^T         = exp(S^T / 8) * causal_mask    (ACT + DVE), bf16
            Yaug^T[n,i] = sum_j v_aug[j, n] A^T[j, i]   (lhsT=v_aug,    rhs=A^T)
              where v_aug has a ones column: row 64 of Yaug^T = softmax denom l
            y^T         = Yaug^T[0:64] * (1/l)          (recip + partition bcast)
  proj:     out[i, o]   = sum_n yT[n, i] WpT[n, o]      (lhsT=yT blk,   rhs=WpT)

v2: single fused pipeline.  The qkv matmuls (pure TensorE) are chopped into
256-column "rounds" and fed into the ACT-bound attention loop by a
deficit-driven feeder, so the scalar engine (exp) and tensor engine stay
concurrently busy instead of running in serial phases.  Each i-chunk's
projection is likewise drained into the next chunk's attention.  Heads are
processed in pairs on partition halves 0-63/64-127 (auto row-tiling packs the
two K=64 S^T matmuls onto disjoint PE row-groups); the pair shares one
[65, 1024] PSUM Y tile so reciprocal/broadcast run once per pair.  exp output
is bf16 (matmul moving operand; ~4e-3 elementwise, averages out in AV).
"""

import numpy as np

import concourse.mybir as mybir
import concourse.tile as tile
from concourse import bacc
from concourse.bass_utils import run_bass_kernel_spmd

F32 = mybir.dt.float32
F32R = mybir.dt.float32r
BF16 = mybir.dt.bfloat16
Exp = mybir.ActivationFunctionType.Exp

B, C, H = 4, 1024, 16
HPC = 8            # heads per core
HD = 64            # head dim
GQ = HPC * HD      # 512 columns per head group
P = 128
KB = C // P        # 8 k-blocks
SCALE = 0.125      # 1/sqrt(HD)
XW = 256           # x round width (columns of T per feeder round)

# dtypes must match within each matmul's (lhsT, rhs) pair: walrus rejects
# mixing 32-bit (f32/f32r) with 16-bit inputs.  Pairs: qkv GEMM (x,w),
# S^T (k,q), AV (v,at), proj (yt,wp).
AGGR = True        # True: bf16 x/w/v/at/yt (S^T stays f32r).  False: bf16 only v/at.
XDT = BF16 if AGGR else F32R   # x, and qkv weights (same GEMM pair)
WDT = XDT
QKDT = BF16        # q, k (S^T pair) bf16: ~7% faster matmuls (probe), err ~doubles
VDT = BF16         # v, at (AV pair)
ADT = BF16         # exp output + mask
YDT = BF16 if AGGR else F32R   # yt, wp (proj pair)

# serial cost-model estimates (ns) used only for feeder pacing
_MM = 0.4167       # ns per streamed matmul column @2.4GHz
_ACT = 1.0 / 1.2   # ns per element-column on ACT
_ACTF = 172 / 1.2 + 57  # fixed per-ACT overhead


def build(T=2048, dup=1, at_bufs=8, ps1_bufs=2, psS_bufs=2, psY_bufs=2):
    nT = T // P        # j-blocks (16)
    nCh = T // 512     # i-chunks (4)
    nX = T // XW       # x rounds (8)
    rpc = 512 // XW    # rounds per i-chunk (2)
    nc = bacc.Bacc("TRN2", target_bir_lowering=False, debug=False, num_devices=8)

    xT = nc.dram_tensor("xT", [C, T], XDT, kind="ExternalInput").ap()
    wqk0T = nc.dram_tensor("wqk0T", [C, 2 * P], WDT, kind="ExternalInput").ap()
    wqkT = nc.dram_tensor("wqkT", [C, 2 * GQ], WDT, kind="ExternalInput").ap()
    wvT = nc.dram_tensor("wvT", [C, GQ], WDT, kind="ExternalInput").ap()
    wpT = nc.dram_tensor("wpT", [GQ, C], YDT, kind="ExternalInput").ap()
    triT = nc.dram_tensor("triT", [P, P], ADT, kind="ExternalInput").ap()
    out = nc.dram_tensor("out", [T, C], BF16, kind="ExternalOutput").ap()

    with tile.TileContext(nc) as tc:
      for _rep in range(dup):
        with tc.tile_pool(name="persist", bufs=1) as pe, \
             tc.tile_pool(name="roll", bufs=2) as roll, \
             tc.tile_pool(name="wrk", bufs=at_bufs) as wrk, \
             tc.tile_pool(name="fin", bufs=1) as fin, \
             tc.tile_pool(name="psS", bufs=psS_bufs, space="PSUM") as psS, \
             tc.tile_pool(name="psY", bufs=psY_bufs, space="PSUM") as psY, \
             tc.tile_pool(name="ps1", bufs=ps1_bufs, space="PSUM") as ps1:

            k_sb = pe.tile([P, 4 * T], QKDT, tag="k")   # 4 nb-blocks (2 heads each)
            v_sb = pe.tile([P, nT * HPC * (HD + 1)], VDT, tag="v")
            wq_sb = pe.tile([P, KB * GQ], WDT, tag="wq")
            wk_sb = pe.tile([P, KB * GQ], WDT, tag="wk")
            wqk0_sb = pe.tile([P, KB * 2 * P], WDT, tag="wqk0")  # nb0 q|k
            wv_sb = pe.tile([P, KB * GQ], WDT, tag="wv")
            wp_sb = pe.tile([P, 4 * C], YDT, tag="wp")
            tri_sb = pe.tile([P, P], ADT, tag="tri")

            # rolling chunked tiles: x rounds, q chunks, y chunks
            x_t = [roll.tile([P, KB * XW], XDT, tag="x", name=f"x{r}", bufs=4)
                   for r in range(nX)]
            q_t = [roll.tile([P, 4 * 512], QKDT, tag="q", name=f"q{ci}")
                   for ci in range(nCh)]
            # per-nb yt tiles: proj's nb-th accumulation step then only
            # depends on pair nb's normalization, not the whole chunk's
            yt_t = [[roll.tile([P, 512], YDT, tag="yt", name=f"yt{ci}_{nb}",
                               bufs=12) for nb in range(4)]
                    for ci in range(nCh)]

            xTr = xT.rearrange("(kb p) t -> p kb t", p=P)

            def xdma(r):
                nc.sync.dma_start(
                    x_t[r][:].rearrange("p (kb w) -> p kb w", kb=KB),
                    xTr[:, :, r * XW:(r + 1) * XW])

            # ---- prologue: no-DMA setup first (act table, v ones column) ----
            scr = fin.tile([P, 2], F32, tag="scr")
            nc.vector.memset(scr[:], 0.0)
            nc.scalar.activation(scr[:], scr[:], Exp)  # preload act table
            if _rep == 0:
                # HAM warm-up: ~3.4us of dummy matmuls riding the initial DMA
                # wait, so the first real matmuls run at K=8/8 instead of
                # 1.2GHz (only in the first body: later dup reps are warm)
                scr2 = fin.tile([P, 512], BF16, tag="scr2")
                nc.vector.memset(scr2[:], 0.0)
                dps = ps1.tile([16, 512], F32, tag="ps1", name="warm")
                for _w in range(8):  # 8 x ~427ns cold spans the ~3.4us window
                    nc.tensor.matmul(dps[:], scr2[:, 0:16], scr2[:],
                                     start=True, stop=True)
            nc.gpsimd.memset(
                v_sb[:].rearrange("p (j h w) -> p j h w", j=nT, h=HPC)[:, :, :, HD:HD + 1],
                1.0)
            # ---- prologue DMAs (ordered by first consumer; wq/wk split so
            # the nb0 feeder units start after ~2us instead of ~9us) ----
            wqkTr = wqkT.rearrange("(kb p) n -> p kb n", p=P)
            wq_r = wq_sb[:].rearrange("p (kb n) -> p kb n", kb=KB)
            wk_r = wk_sb[:].rearrange("p (kb n) -> p kb n", kb=KB)
            # kb0 slivers first: the first qk matmul can issue ~2.5us earlier
            wqk0r = wqk0T.rearrange("(kb p) n -> p kb n", p=P)
            nc.sync.dma_start(wqk0_sb[:, 0:2 * P], wqk0r[:, 0, :])
            nc.sync.dma_start(
                x_t[0][:].rearrange("p (kb w) -> p kb w", kb=KB)[:, 0:1, :],
                xTr[:, 0:1, 0:XW])
            nc.sync.dma_start(
                wqk0_sb[:].rearrange("p (kb n) -> p kb n", kb=KB)[:, 1:4, :],
                wqk0r[:, 1:4, :])
            nc.sync.dma_start(
                x_t[0][:].rearrange("p (kb w) -> p kb w", kb=KB)[:, 1:4, :],
                xTr[:, 1:4, 0:XW])
            nc.sync.dma_start(
                wqk0_sb[:].rearrange("p (kb n) -> p kb n", kb=KB)[:, 4:, :],
                wqk0r[:, 4:, :])
            nc.sync.dma_start(
                x_t[0][:].rearrange("p (kb w) -> p kb w", kb=KB)[:, 4:, :],
                xTr[:, 4:, 0:XW])
            xdma(1)
            nc.sync.dma_start(
                wv_sb[:].rearrange("p (kb n) -> p kb n", kb=KB),
                wvT.rearrange("(kb p) n -> p kb n", p=P))
            nc.sync.dma_start(wq_r[:, :, P:GQ], wqkTr[:, :, P:GQ])
            nc.sync.dma_start(wk_r[:, :, P:GQ], wqkTr[:, :, GQ + P:2 * GQ])
            xdma(2)
            xdma(3)
            nc.sync.dma_start(tri_sb[:], triT)
            nc.sync.dma_start(
                wp_sb[:].rearrange("p (kb n) -> p kb n", kb=4),
                wpT.rearrange("(kb p) n -> p kb n", p=P))

            # ---- phase-1 units ----
            def qk_unit(r, nb, wsel, dst, dcol):
                # dst[:, dcol:dcol+XW] = (w block nb).T @ x round r
                def emit():
                    pt = ps1.tile([P, XW], F32, tag="ps1", name=f"p1_{r}_{nb}")
                    for kb in range(KB):
                        if nb == 0:  # nb0 lives in the fast-start wqk0 tile
                            o = kb * 2 * P + (0 if wsel == "q" else P)
                            w_ap = wqk0_sb[:, o:o + P]
                        else:
                            w_sb = wq_sb if wsel == "q" else wk_sb
                            w_ap = w_sb[:, kb * GQ + nb * P:
                                        kb * GQ + (nb + 1) * P]
                        nc.tensor.matmul(
                            pt[:], w_ap,
                            x_t[r][:, kb * XW:(kb + 1) * XW],
                            start=(kb == 0), stop=(kb == KB - 1))
                    nc.vector.tensor_copy(dst[:, dcol:dcol + XW], pt[:])
                return emit, KB * XW * _MM

            def v_unit(r, jj):
                jb = (r * XW) // P + jj
                def emit():
                    pt = ps1.tile([P, GQ], F32, tag="ps1", name=f"pv_{r}_{jj}")
                    for kb in range(KB):
                        nc.tensor.matmul(
                            pt[:],
                            x_t[r][:, kb * XW + jj * P: kb * XW + (jj + 1) * P],
                            wv_sb[:, kb * GQ:(kb + 1) * GQ],
                            start=(kb == 0), stop=(kb == KB - 1))
                    vv = v_sb[:, jb * HPC * (HD + 1):(jb + 1) * HPC * (HD + 1)] \
                        .rearrange("p (h w) -> p h w", h=HPC)
                    nc.vector.tensor_copy(vv[:, :, 0:HD],
                                          pt[:].rearrange("p (h w) -> p h w", h=HPC))
                return emit, KB * GQ * _MM

            # feeder: [(deadline, emit, pe_ns), ...] sorted by deadline.
            # deadline d means: must be emitted before attention pair
            # (ci=floor(d), hp=2*int(4*frac(d))) starts.
            feeder = []
            for c in range(nCh):
                r0, r1 = 2 * c, 2 * c + 1
                for nb in range(4):
                    for r in (r0, r1):
                        off = (r * XW) % 512
                        feeder.append(
                            (c + nb / 4,)
                            + qk_unit(r, nb, "q", q_t[c], nb * 512 + off))
                        feeder.append(
                            (c + nb / 4,)
                            + qk_unit(r, nb, "k", k_sb, nb * T + r * XW))
                    if nb == 0:  # v due at pair 0's av, just after nb0 q/k
                        for r in (r0, r1):
                            for jj in range(XW // P):
                                feeder.append((float(c),) + v_unit(r, jj))
                for r in (r0, r1):
                    if r + 4 < nX:
                        feeder.append(
                            (c + 0.8 + (r % 2) / 20, (lambda rr=r + 4: xdma(rr)), 0.0))
            fpos = [0]

            def drain_feeder(max_d, deficit=None):
                # deficit None -> force-drain everything with deadline <= max_d
                d = deficit
                while fpos[0] < len(feeder):
                    dl, emit, cost = feeder[fpos[0]]
                    if dl > max_d:
                        break
                    if d is not None:
                        if d < cost:
                            break
                        d -= cost
                    emit()
                    fpos[0] += 1
                return 0.0 if d is None else d

            # ---- proj units (chunk ci) ----
            def proj_unit(ci, mb):
                split_dma = mb >= 4 * nCh - 2  # last units: overlap the tail
                def emit():
                    ot = wrk.tile([P, 1024], BF16, tag="ot", bufs=3, name=f"ot{mb}")
                    for oc in range(2):
                        po_ = ps1.tile([P, 512], F32, tag="ps1", name=f"po{mb}_{oc}")
                        for nb in range(4):
                            nc.tensor.matmul(
                                po_[:],
                                yt_t[ci][nb][:, (mb - 4 * ci) * P:
                                             (mb - 4 * ci + 1) * P],
                                wp_sb[:, nb * C + oc * 512: nb * C + (oc + 1) * 512],
                                start=(nb == 0), stop=(nb == 3))
                        if mb >= 4 * nCh - 5:
                            # endgame: ACT is idle and DVE is the tail's
                            # critical path -- evacuate PSUM via ScalarE
                            nc.scalar.copy(ot[:, oc * 512:(oc + 1) * 512], po_[:])
                        else:
                            nc.vector.tensor_copy(ot[:, oc * 512:(oc + 1) * 512],
                                                  po_[:])
                        if split_dma:
                            nc.sync.dma_start(
                                out[mb * P:(mb + 1) * P, oc * 512:(oc + 1) * 512],
                                ot[:, oc * 512:(oc + 1) * 512])
                    if not split_dma:
                        nc.sync.dma_start(out[mb * P:(mb + 1) * P, :], ot[:])
                return emit, 2 * 4 * 512 * _MM

            proj_q = []
            norm_q = []

            def drain_norms():
                # the scale multiply runs on GpSimd (otherwise idle), keeping
                # the DVE queue free for mask-muls/copies that gate the PE
                for ci_, po, qc, src, rb, on_dve in norm_q:
                    dst = yt_t[ci_][qc][po:po + HD, :]
                    if on_dve:
                        nc.vector.tensor_mul(dst, src, rb[:])
                    else:
                        nc.gpsimd.tensor_mul(dst, src, rb[:])
                norm_q.clear()

            allow_proj = [True]

            def pull(deficit):
                # drain due PE work: proj backlog first, then feeder (capped).
                # proj is held back during a chunk's first pair (its yt deps
                # land mid-pair and would stall the in-order PE queue), and
                # one unit is kept for the chunk boundary, where it gives the
                # PE dep-free work while the next chunk's norms/DMAs land.
                while allow_proj[0] and len(proj_q) > 1 and deficit >= proj_q[0][1]:
                    e, c = proj_q.pop(0)
                    e()
                    deficit -= c
                return drain_feeder(ci + 1 + 7 / 8, deficit)

            # ---- fused attention loop ----
            for ci in range(nCh):
                jfull = 4 * ci
                jmax = jfull + 4
                deficit = 0.0
                for hp in range(0, HPC, 2):
                    allow_proj[0] = hp >= 2
                    drain_feeder(ci + hp / 8)     # hard deps for this pair
                    drain_norms()                 # previous pair's softmax div
                    if hp == 0 and ci >= 2 and proj_q:
                        # held-back proj unit: dep-free PE work that bridges
                        # the chunk boundary's norm/exp latency (not at ci=1:
                        # chunk 0's norms are still in flight there)
                        proj_q.pop(0)[0]()
                    # pro-rata quota: spread units due by the NEXT pair across
                    # this pair's iterations instead of dumping at its start
                    d_next = ci + (hp + 2) / 8
                    base = fpos[0]
                    m = 0
                    while base + m < len(feeder) and feeder[base + m][0] <= d_next:
                        m += 1
                    n_it = 2 * ci + 4
                    it = [0]

                    def quota_drain():
                        it[0] += 1
                        want = base + min(m, -(-m * it[0] // n_it))
                        while fpos[0] < want:
                            feeder[fpos[0]][1]()
                            fpos[0] += 1
                    hs = (hp, hp + 1)
                    st = {h: dict(po=(h % 2) * HD, qc=(h // 2), vc=h * (HD + 1),
                                  py=psY.tile([HD + 1, 512], F32, tag="psY",
                                              name=f"py{ci}_{h}"))
                          for h in hs}

                    def st_mm(h, dst, jb, a):
                        s = st[h]
                        nc.tensor.matmul(
                            dst,
                            k_sb[s["po"]:s["po"] + HD,
                                 s["qc"] * T + jb * P: s["qc"] * T + (jb + 1) * P],
                            q_t[ci][s["po"]:s["po"] + HD,
                                    s["qc"] * 512 + a: (s["qc"] + 1) * 512],
                            start=True, stop=True)

                    def av_mm(h, jb, at_ap, a):
                        s = st[h]
                        nc.tensor.matmul(
                            s["py"][:, a:512],
                            v_sb[:, jb * HPC * (HD + 1) + s["vc"]:
                                 jb * HPC * (HD + 1) + s["vc"] + HD + 1],
                            at_ap,
                            start=(jb == 0), stop=(jb == jmax - 1))

                    for j0 in range(0, jfull, 2):      # full blocks, paired
                        ats = {}
                        for h in hs:
                            psp = psS.tile([P, 1024], F32, tag="psS",
                                           name=f"psp{ci}_{h}_{j0}")
                            st_mm(h, psp[:, 0:512], j0, 0)
                            st_mm(h, psp[:, 512:1024], j0 + 1, 0)
                            at = wrk.tile([P, 1024], ADT, tag="at",
                                          name=f"at{ci}_{h}_{j0}")
                            nc.scalar.activation(at[:], psp[:], Exp, scale=SCALE)
                            ats[h] = at
                        deficit += 2 * (1024 * _ACT + _ACTF) - 8 * 512 * _MM
                        deficit = pull(deficit)
                        quota_drain()
                        for h in hs:
                            av_mm(h, j0, ats[h][:, 0:512], 0)
                            av_mm(h, j0 + 1, ats[h][:, 512:1024], 0)

                    for p_ in range(4):                # crossing blocks, 2 heads packed
                        jb = jfull + p_
                        a = P * p_
                        w = 512 - a
                        psp = psS.tile([P, 1024], F32, tag="psS",
                                       name=f"psx{ci}_{hp}_{p_}")
                        st_mm(hs[0], psp[:, a:512], jb, a)
                        st_mm(hs[1], psp[:, 512:512 + w], jb, a)
                        at = wrk.tile([P, 1024], ADT, tag="at",
                                      name=f"atx{ci}_{hp}_{p_}")
                        nc.scalar.activation(at[:, a:512 + w], psp[:, a:512 + w],
                                             Exp, scale=SCALE)
                        # only the 128-wide diagonal block of each head needs
                        # the causal mask; everything else is fully unmasked
                        nc.vector.tensor_mul(at[:, a:a + P], at[:, a:a + P],
                                             tri_sb[:])
                        nc.vector.tensor_mul(at[:, 512:512 + P],
                                             at[:, 512:512 + P], tri_sb[:])
                        deficit += (2 * w * _ACT + _ACTF) - 4 * w * _MM
                        deficit = pull(deficit)
                        quota_drain()
                        av_mm(hs[0], jb, at[:, a:512], a)
                        av_mm(hs[1], jb, at[:, 512:512 + w], a)

                    # reciprocals first (tiny PSUM read, kicks off the Pool
                    # broadcast chain early), then evacuate Yaug to SBUF
                    # (frees the PSUM bank); the broadcast/mul finish in the
                    # next pair so they never gate this boundary's PSUM reuse
                    rbs = {}
                    for h in hs:
                        rt = fin.tile([1, 512], F32, tag="rt", bufs=4,
                                      name=f"rt{ci}_{h}")
                        nc.vector.reciprocal(rt[:], st[h]["py"][HD:HD + 1, :])
                        rb = fin.tile([HD, 512], F32, tag="rb", bufs=4,
                                      name=f"rb{ci}_{h}")
                        nc.gpsimd.partition_broadcast(rb[:], rt[:])
                        rbs[h] = rb
                    last = ci == nCh - 1 and hp == HPC - 2
                    for h in hs:
                        if last:
                            # no successor pair needs this PSUM bank: let the
                            # DVE mul read Yaug in place, skip the copy
                            norm_q.append((ci, st[h]["po"], st[h]["qc"],
                                           st[h]["py"][0:HD, :], rbs[h], True))
                            continue
                        ya = wrk.tile([HD + 1, 512], F32, tag="ya", bufs=6,
                                      name=f"ya{ci}_{h}")
                        nc.vector.tensor_copy(ya[0:HD, :], st[h]["py"][0:HD, :])
                        norm_q.append((ci, st[h]["po"], st[h]["qc"],
                                       ya[0:HD, :], rbs[h], False))

                # drain the previous chunk's proj down to one held unit (the
                # boundary filler), then queue this chunk's; the last pair's
                # norms drain at the next pair top, before any proj pull
                while len(proj_q) > 1:
                    proj_q.pop(0)[0]()
                proj_q = proj_q + [proj_unit(ci, mb)
                                   for mb in range(4 * ci, 4 * ci + 4)]

            drain_norms()
            drain_feeder(float(nCh))
            for e, _c in proj_q:
                e()
    return nc


_CACHE = {}


def get_nc(T=2048):
    if T not in _CACHE:
        nc = build(T)
        nc.compile()
        _CACHE[T] = nc
    return _CACHE[T]


def make_in_maps(x, W_attn, W_proj):
    Bx, T, Cx = x.shape
    Wq, Wk, Wv = W_attn[:Cx], W_attn[Cx:2 * Cx], W_attn[2 * Cx:]
    import ml_dtypes
    cva = lambda a: np.ascontiguousarray(a).astype(ml_dtypes.bfloat16)
    cv = cva if AGGR else (lambda a: np.ascontiguousarray(a))
    cvw = cva if AGGR else (lambda a: np.ascontiguousarray(a))
    cvp = cva if AGGR else (lambda a: np.ascontiguousarray(a))
    r = np.arange(P)
    tri = (r[:, None] <= r[None, :]).astype(np.float32)
    in_maps = []
    for core in range(8):
        b, g = divmod(core, 2)
        rows = slice(g * GQ, (g + 1) * GQ)
        in_maps.append({
            "xT": cv(x[b].T),
            "wqk0T": cvw(np.concatenate([Wq[rows][:P], Wk[rows][:P]], 0).T),
            "wqkT": cvw(np.concatenate([Wq[rows], Wk[rows]], 0).T),
            "wvT": cvw(Wv[rows].T),
            "wpT": cvp(W_proj[:, rows].T),
            "triT": cva(tri),
        })
    return in_maps


def kernel(x, W_attn, W_proj):
    x = np.asarray(x, dtype=np.float32)
    W_attn = np.asarray(W_attn, dtype=np.float32)
    W_proj = np.asarray(W_proj, dtype=np.float32)
    Bx, T, Cx = x.shape
    assert (Bx, Cx) == (B, C) and W_attn.shape == (3 * C, C) and W_proj.shape == (C, C)
    nc = get_nc(T)
    res = run_bass_kernel_spmd(nc, make_in_maps(x, W_attn, W_proj), list(range(8)))
    out = np.empty((Bx, T, Cx), np.float32)
    for b in range(Bx):
        out[b] = (res.results[2 * b]["out"].astype(np.float32)
                  + res.results[2 * b + 1]["out"].astype(np.float32))
    return out


if __name__ == "__main__":
    rng = np.random.default_rng(0)
    x = rng.standard_normal((B, 2048, C), dtype=np.float32)
    W_attn = rng.standard_normal((3 * C, C), dtype=np.float32) * (1.0 / np.sqrt(C))
    W_proj = rng.standard_normal((C, C), dtype=np.float32) * (1.0 / np.sqrt(C))
    out = kernel(x, W_attn, W_proj)
    print("out", out.shape, out.dtype, np.abs(out).max())

